# revision 82
# baseline (speedup 1.0000x reference)
"""MultiHeadAttention Trainium2 kernel (8 NeuronCores, SPMD).

Sharding: data-parallel over batch (B=2), tensor-parallel over heads
(16 heads -> 4 per core).  Core c handles batch b=c//4, head group
g=c%4 (heads 4g..4g+3).  Wq/Wk/Wv are split column-wise, Wo row-wise;
the per-core Wo partial outputs are summed on the host (replaces the
all-reduce).

Device dataflow per core (bf16 matmuls, f32 PSUM accumulation):
  qT = Wq_g^T x^T   [256, 2048]   (heads on partitions, dh=64 each)
  kT = Wk_g^T x^T   [256, 2048]
  v  = x Wv_g       [2048, 256] stored interleaved with a ones column
                    per head: vaug[st] = [vA|1|vB|1|vC|1|vD|1]
  per (s_q chunk of 512, head pair):
    logitsT[s_k, s_q] = kT^T qT / 8       (two heads packed in PE row
                                           groups, K=64 each)
    p = exp(logitsT)  on ScalarE, scale=1/8 fused, bf16 out
    accT[65, s_q] += vaug_h^T p           (row 64 = softmax denominator)
    outcatT[h] = accT[0:64] * bcast(1/accT[64])   (deferred softmax norm)
  partial = outcatT^T Wo_g  -> DRAM bf16 (summed in f32 on the host)

The kernel is organized as one flat software pipeline: the exp stream
on ScalarE is the pacer (~1.15us per (c,pr,st) step); everything else
(projections, Wo, output DMA) is deadline-scheduled into the PE slack
under it.  Inputs are loaded in 512-column chunks (one 3D DMA
descriptor per chunk) so the first exp fires after ~4.5MB of DMA
instead of the full 14.7MB.
"""

import itertools
import sys

import numpy as np

sys.path.insert(0, "/opt/trn_rl_repo")

import ml_dtypes  # noqa: E402

import concourse.bass as bass  # noqa: E402
import concourse.mybir as mybir  # noqa: E402
import concourse.tile as tile  # noqa: E402
from concourse import bacc  # noqa: E402
from concourse.bass import ts  # noqa: E402
from concourse.bass_utils import run_bass_kernel_spmd  # noqa: E402

S = 2048  # sequence length (S * X)
D = 1024  # model dim
H = 16  # total heads
HL = 4  # heads per core
DH = 64  # head dim
DQ = HL * DH  # per-core projection width = 256
NK = D // 128  # K tiles for projections = 8
NST = S // 128  # s_k tiles = 16
NCH = S // 512  # s_q chunks = 4
LAG = 7  # front-to-accumulate pipeline distance (runahead for stalls)

BF16 = mybir.dt.bfloat16
F32 = mybir.dt.float32

TRACE = False
LAST_RESULTS = None

_BUILT = None


def _emit(ctx, tc, io):
    nc = tc.nc
    xq, xk, xv = io["xqT"], io["xkT"], io["xvT"]
    wq, wk, wv, wo = io["wq"], io["wk"], io["wv"], io["wo"]
    bq, bk, bv = io["bq"], io["bk"], io["bv"]
    out = io["out"]

    consts = ctx.enter_context(tc.tile_pool(name="consts", bufs=1))
    xin = ctx.enter_context(tc.tile_pool(name="xin", bufs=1))
    qk = ctx.enter_context(tc.tile_pool(name="qk", bufs=1))
    ptiles = ctx.enter_context(tc.tile_pool(name="ptiles", bufs=12))
    norm = ctx.enter_context(tc.tile_pool(name="norm", bufs=3))
    osb_pool = ctx.enter_context(tc.tile_pool(name="osb", bufs=4))
    psum_mm = ctx.enter_context(tc.tile_pool(name="psum_mm", bufs=4, space="PSUM"))
    psum_lg = ctx.enter_context(tc.tile_pool(name="psum_lg", bufs=2, space="PSUM"))

    # x and W live as single 3D tiles: [128, k_tile, cols].  One DMA
    # descriptor loads a 512-column chunk of all 8 k-tiles at once (the
    # per-dma_start enqueue cost on the issuing engine is ~650ns, so
    # descriptor count is what paces the input stream).
    wq_all = consts.tile([128, NK, DQ], BF16, tag="wq", name="wq_all")
    wk_all = consts.tile([128, NK, DQ], BF16, tag="wk", name="wk_all")
    wv_all = consts.tile([128, NK, DQ], BF16, tag="wv", name="wv_all")
    wo_all = consts.tile([128, 2, D], BF16, tag="wo", name="wo_all")
    xq_all = xin.tile([128, NK, S], BF16, tag="xq", name="xq_all")
    xk_all = xin.tile([128, NK, S], BF16, tag="xk", name="xk_all")
    xv_all = xin.tile([128, NK, S], BF16, tag="xv", name="xv_all")
    # bq/bk as [128, 2] per-partition scalars (col j = dq 128j..128j+127)
    bq_sb = consts.tile([128, 2], F32, tag="bq", name="bq_sb")
    bk_sb = consts.tile([128, 2], F32, tag="bk", name="bk_sb")
    bv_sb = consts.tile([128, DQ], F32, tag="bv", name="bv_sb")

    # PE warmup: dummy back-to-back matmuls at t=0, while the PE would
    # otherwise sit idle waiting for input DMA.  The PE clock gate (HAM)
    # defaults to 4/8 throttle (1.2 GHz) and only releases after ~3.4us
    # of sustained activity; without this the whole DMA-paced front runs
    # at half clock.
    wu_sb = consts.tile([128, 512], BF16, tag="wu", name="wu_sb")
    nc.vector.memset(wu_sb[:], 1.0)
    wu_ps = psum_lg.tile([128, 512], F32, tag="lg", name="lg")

    for _ in range(48):
        nc.tensor.matmul(wu_ps[:], wu_sb[:, 0:128], wu_sb[:], start=True, stop=True)

    # ---- input DMA.  Host arrays are pre-packed chunk-contiguous
    # ([128 part, k, cols] blocks) so every transfer reads 4-8KB
    # sequential HBM runs per partition.  Three descriptor queues (two
    # HWDGE + gpsimd SWDGE) stream in arrival-priority order: the first
    # (c0, pr0) attention sweep consumes ALL xk chunks plus xq c0/xv c0,
    # so xk is spread across both HWDGE queues ahead of everything else.
    # biases are host-pre-packed to partition-major layouts so these are
    # single-descriptor contiguous transfers (4-byte-run patterns choke
    # the ring for ~7us each)
    nc.sync.dma_start(
        out=bk_sb[:], in_=bass.AP(tensor=bk.tensor, offset=bk.offset, ap=[[2, 128], [1, 2]])
    )
    nc.sync.dma_start(
        out=bq_sb[:], in_=bass.AP(tensor=bq.tensor, offset=bq.offset, ap=[[2, 128], [1, 2]])
    )

    def dma_w(q, dst, src, nkt, cols):
        q.dma_start(
            out=dst[:],
            in_=bass.AP(
                tensor=src.tensor,
                offset=src.offset,
                ap=[[nkt * cols, 128], [cols, nkt], [1, cols]],
            ),
        )

    def dma_chunk(q, dst, src, c):
        q.dma_start(
            out=dst[:, :, ts(c, 512)],
            in_=bass.AP(
                tensor=src.tensor,
                offset=src.offset + c * (128 * NK * 512),
                ap=[[NK * 512, 128], [512, NK], [1, 512]],
            ),
        )

    def dma_half(q, dst, src, c, half):
        q.dma_start(
            out=dst[:, :, c * 512 + half * 256 : c * 512 + half * 256 + 256],
            in_=bass.AP(
                tensor=src.tensor,
                offset=src.offset + c * (128 * NK * 512) + half * 256,
                ap=[[NK * 512, 128], [512, NK], [1, 256]],
            ),
        )

    # Measured DMA behavior: aggregate input bandwidth is hard-capped at
    # ~320-360 GB/s/core; the gpsimd SWDGE stream sustains ~210 GB/s on
    # its own (serial, ucode sprays hw rings) while each HWDGE ring
    # (sync / scalar engines) only gets ~40-50 GB/s under contention.
    # So the arrival-critical chain (xk0, xq0, then the mid-sweep
    # chunks) rides the fast SWDGE stream in consumption order; the
    # 0.5MB weights and latency-tolerant late chunks go on the rings.
    nc.sync.dma_start(
        out=bv_sb[:], in_=bass.AP(tensor=bv.tensor, offset=bv.offset, ap=[[DQ, 128], [1, DQ]])
    )
    dma_chunk(nc.sync, xv_all, xv, 1)
    dma_chunk(nc.sync, xq_all, xq, 1)
    dma_w(nc.scalar, wq_all, wq, NK, DQ)
    dma_chunk(nc.scalar, xk_all, xk, 1)
    dma_chunk(nc.scalar, xv_all, xv, 2)
    dma_chunk(nc.scalar, xq_all, xq, 2)
    dma_w(nc.gpsimd, wk_all, wk, NK, DQ)
    dma_chunk(nc.gpsimd, xk_all, xk, 0)
    dma_chunk(nc.gpsimd, xq_all, xq, 0)
    dma_chunk(nc.gpsimd, xv_all, xv, 0)
    dma_w(nc.gpsimd, wv_all, wv, NK, DQ)
    dma_chunk(nc.gpsimd, xk_all, xk, 2)
    dma_chunk(nc.gpsimd, xk_all, xk, 3)
    dma_chunk(nc.gpsimd, xv_all, xv, 3)
    dma_chunk(nc.gpsimd, xq_all, xq, 3)
    dma_w(nc.gpsimd, wo_all, wo, 2, D)

    # ---- projections: qT, kT = [256, 2048] as 2 tiles of [128, 2048] ----
    qT = [qk.tile([128, S], BF16, tag=f"qT{m}", name=f"qT{m}") for m in range(2)]
    kT = [qk.tile([128, S], BF16, tag=f"kT{m}", name=f"kT{m}") for m in range(2)]

    def qk_quarter(w_all, dst, b_sb, m, c, on_act, state, q):
        """Quarter of a q/k projection PSUM group (k pair 2q, 2q+1).

        Quarters are ~0.43us PE lumps so they interleave between attention
        steps without stalling the exp feed (the front can only run ~2
        steps ahead of the exp stream via the psum_lg double buffer).  The
        PSUM group stays open between quarters; unrelated matmuls to other
        banks interleave freely, but the scheduler bounds how many other
        psum_mm allocations occur while a group is open (pool slot reuse
        of an open group would deadlock the PE queue).
        """
        x_all = xq_all if dst is qT else xk_all
        if q == 0:
            state["ps"] = psum_mm.tile([128, 512], F32, tag="mm", name="mm")
        ps = state["ps"]
        for k in (2 * q, 2 * q + 1):
            nc.tensor.matmul(
                ps[:],
                w_all[:, k, ts(m, 128)],
                x_all[:, k, ts(c, 512)],
                start=(k == 0),
                stop=(k == NK - 1),
            )
        if q == 3:
            if on_act:  # prefix phase: ScalarE is idle there
                nc.scalar.add(dst[m][:, ts(c, 512)], ps[:], b_sb[:, m : m + 1])
            else:  # injected into attention: keep ScalarE free for exp
                nc.vector.tensor_scalar_add(dst[m][:, ts(c, 512)], ps[:], b_sb[:, m : m + 1])

    # vaug[st] = [vA|1|vB|1|vC|1|vD|1]  [128, 260]
    vaug = [qk.tile([128, HL * (DH + 1)], BF16, tag=f"vaug{st}", name=f"vaug{st}") for st in range(NST)]

    # the ones-columns of every vaug tile are set up front, during the
    # DMA-wait window where the DVE is idle (the per-head adds below
    # only write the value columns)
    for st in range(NST):
        nc.vector.memset(vaug[st][:], 1.0)

    def v_half(st, state, h):
        """Half of the v projection for one st tile (k 0-3 or 4-7)."""
        if h == 0:
            state["ps"] = psum_mm.tile([128, DQ], F32, tag="mm", name="mm")
        ps = state["ps"]
        for k in range(4 * h, 4 * h + 4):
            nc.tensor.matmul(
                ps[:],
                xv_all[:, k, ts(st, 128)],
                wv_all[:, k, :],
                start=(k == 0),
                stop=(k == NK - 1),
            )
        if h == 1:
            for hd in range(HL):
                nc.vector.tensor_add(
                    vaug[st][:, hd * 65 : hd * 65 + 64],
                    ps[:, ts(hd, DH)],
                    bv_sb[:, ts(hd, DH)],
                )

    octT = [qk.tile([128, S], BF16, tag=f"octT{m}", name=f"octT{m}") for m in range(2)]
    store_q = itertools.cycle([nc.sync, nc.gpsimd])
    tail_store_q = itertools.cycle([nc.sync, nc.scalar])
    tail_cp = itertools.cycle([True, False])  # alternate scalar/vector copies

    tail_mode = {"on": False}  # set once the exp stream is fully drained
    wo_psum = itertools.cycle([0, 1, 0])  # tail: spread over mm + lg banks

    def wo_group(c, smt, ncho, on_act=False):
        row = c * 512 + smt * 128
        if tail_mode["on"] and next(wo_psum):
            # the exp stream is done, so the logits banks are free: using
            # them decouples the final Wo matmuls from the copy drain
            ps = psum_lg.tile([128, 512], F32, tag="lg", name="lg")
        else:
            ps = psum_mm.tile([128, 512], F32, tag="mm", name="mm")
        for k in range(2):
            nc.tensor.matmul(
                ps[:],
                octT[k][:, row : row + 128],
                wo_all[:, k, ts(ncho, 512)],
                start=(k == 0),
                stop=(k == 1),
            )
        osb = osb_pool.tile([128, 512], BF16, tag="osb", name="osb")
        if (on_act or tail_mode["on"]) and next(tail_cp):
            # tail drain: split copies between the idle ScalarE and DVE
            nc.scalar.copy(osb[:], ps[:])
        else:
            nc.vector.tensor_copy(osb[:], ps[:])
        # tail stores avoid the SWDGE queue: its end-of-kernel drain
        # (~3us) starts only after its last descriptor completes.  They
        # alternate across both HWDGE rings so the final burst drains in
        # parallel.
        q = next(tail_store_q) if tail_mode["on"] else next(store_q)
        q.dma_start(out[row : row + 128, ts(ncho, 512)], osb[:])

    # ---- prefix: just enough for the first exp: kT/qT chunk 0 of the
    # pr0 pair.  Everything else is scheduled into the attention
    # pipeline below, one ~0.43us quarter at a time, ordered by DMA
    # arrival estimate and forced by consumption deadline.
    st0 = {}
    for q in range(4):
        qk_quarter(wk_all, kT, bk_sb, 0, 0, True, st0, q)
    # keep the PE clock gate open while the qT prefix waits for xq0/wq
    for _ in range(18):
        nc.tensor.matmul(wu_ps[:, 0:256], wu_sb[:, 0:128], wu_sb[:, 0:256], start=True, stop=True)
    st1 = {}
    for q in range(4):
        qk_quarter(wq_all, qT, bq_sb, 0, 0, True, st1, q)

    steps = [(c, pr, st) for c in range(NCH) for pr in range(2) for st in range(NST)]

    # Arrival estimates (us from kernel start) for each input buffer,
    # from the queue FIFOs above: SWDGE ~210 GB/s serial starting ~7.5us,
    # HWDGE rings ~70 GB/s each.  The step->time map is piecewise: the
    # first two sweeps are dense with forced projection units (~1.7
    # us/step), after which the pipeline cruises at ~1.15 us/step.
    T0_US = 23.0
    ARR_XK = [19.0, 26.0, 38.0, 42.0]
    ARR_XQ = [24.0, 47.0, 56.0, 52.0]
    ARR_XV = [29.5, 33.0, 41.0, 47.0]

    def astep(a_us):
        t = a_us - T0_US
        if t <= 0:
            return 0
        if t <= 32 * 1.75:
            return int(t / 1.75) + 1
        return 32 + int((t - 32 * 1.75) / 1.15) + 1

    # Work groups: each is a sequence of ~0.43us PE units sharing one
    # open psum_mm accumulation group.  rel = earliest emission step
    # (DMA arrival), need = step whose front/acc consumes the output.
    groups = []

    def add_group(rel_us, need, units):
        groups.append({"rel": astep(rel_us), "need": need, "units": units, "idx": 0})

    def qk_units(w_all, dst, b_sb, m, c):
        state = {}
        return [
            (lambda w=w_all, d=dst, b=b_sb, mm=m, cc=c, s=state, q=q: qk_quarter(w, d, b, mm, cc, False, s, q))
            for q in range(4)
        ]

    def v_units(st):
        state = {}
        return [(lambda s=st, ss=state, h=h: v_half(s, ss, h)) for h in range(2)]

    for st in range(NST):  # vaug[st] consumed at acc step st (= iter st+LAG)
        add_group(ARR_XV[st // 4], st + LAG, v_units(st))
    for c in range(1, NCH):  # kT m0 chunk c consumed at front step 4c
        add_group(ARR_XK[c], 4 * c, qk_units(wk_all, kT, bk_sb, 0, c))
    add_group(ARR_XQ[0], 16, qk_units(wq_all, qT, bq_sb, 1, 0))
    for c in range(NCH):  # kT m1 chunk c consumed at front step 16+4c
        add_group(ARR_XK[c], 16 + 4 * c, qk_units(wk_all, kT, bk_sb, 1, c))
    for c in range(1, NCH):  # qT m chunk c consumed from front step 32c+16m
        add_group(ARR_XQ[c], 32 * c, qk_units(wq_all, qT, bq_sb, 0, c))
        add_group(ARR_XQ[c], 32 * c + 16, qk_units(wq_all, qT, bq_sb, 1, c))
    groups.sort(key=lambda g: g["need"])

    wo_q = []
    acc_map = {}
    p_map = {}

    def emit_front(c, pr, st):
        lg = psum_lg.tile([128, 1024], F32, tag="lg", name="lg")
        for hh in range(2):
            nc.tensor.matmul(
                lg[:, ts(hh, 512)],
                kT[pr][ts(hh, 64), ts(st, 128)],
                qT[pr][ts(hh, 64), ts(c, 512)],
                start=True,
                stop=True,
            )
        p = ptiles.tile([128, 1024], BF16, tag="p", name="p")
        nc.scalar.activation(p[:], lg[:], mybir.ActivationFunctionType.Exp, scale=0.125)
        p_map[(c, pr, st)] = p

    def emit_acc(c, pr, st):
        if st == 0:
            acc_map[(c, pr)] = [
                psum_mm.tile([65, 512], F32, tag="mm", name="mm") for _ in range(2)
            ]
        acc = acc_map[(c, pr)]
        pp = p_map.pop((c, pr, st))
        for hh in range(2):
            h = 2 * pr + hh
            nc.tensor.matmul(
                acc[hh][:],
                vaug[st][:, h * 65 : h * 65 + 65],
                pp[:, ts(hh, 512)],
                start=(st == 0),
                stop=(st == NST - 1),
            )
        if st == NST - 1:
            # normalize: octT[pr][64*hh, chunk c] = acc[0:64] / acc[64].
            # Reciprocal on the [1,512] denominator row, gpsimd broadcast
            # to 64 partitions, DVE muls straight out of acc PSUM.
            final = c == NCH - 1 and pr == 1
            bcs = []
            for hh in range(2):
                den = norm.tile([1, 512], F32, tag="den", name="den")
                if final and hh == 1:
                    # the exp stream is complete by now; ScalarE can take
                    # one den copy so the two chains start in parallel
                    nc.scalar.copy(den[:], acc[hh][64:65, :])
                else:
                    nc.vector.tensor_copy(den[:], acc[hh][64:65, :])
                rden = norm.tile([1, 512], F32, tag="rden", name="rden")
                nc.vector.reciprocal_approx_fast(rden[:], den[:])
                bc = norm.tile([64, 512], F32, tag="bcs", name="bcs")
                nc.gpsimd.partition_broadcast(bc[:], rden[:])
                bcs.append(bc)
            if final:
                # PE fill during the reciprocal/broadcast chain: drain the
                # held-back Wo backlog (also keeps the clock gate open for
                # the final Wo burst below).  The exp stream is complete
                # here: ScalarE takes copies and the lg banks join the
                # Wo psum rotation.
                tail_mode["on"] = True
                while wo_q:
                    wo_q.pop(0)()
                for smt in range(4):
                    for hh in range(2):
                        nc.vector.tensor_mul(
                            octT[pr][ts(hh, 64), c * 512 + smt * 128 : c * 512 + smt * 128 + 128],
                            acc[hh][0:64, ts(smt, 128)],
                            bcs[hh][:, ts(smt, 128)],
                        )
                    wo_group(c, smt, 0, on_act=True)
                    wo_group(c, smt, 1, on_act=True)
            else:
                for hh in range(2):
                    nc.vector.tensor_mul(
                        octT[pr][ts(hh, 64), ts(c, 512)], acc[hh][0:64, :], bcs[hh][:]
                    )
            del acc_map[(c, pr)]
            if pr == 1 and not final:
                wo_q.extend(
                    (lambda cc=c, smt=smt, ncho=ncho: wo_group(cc, smt, ncho))
                    for smt in range(4)
                    for ncho in range(2)
                )

    # Scheduler state: `active` is the group currently being dribbled
    # out one unit per step.  While a psum group is open we emit ONLY
    # its units (no other psum_mm allocations), and we never open a
    # 4-unit group within 3 steps of a sweep boundary (emit_acc st==0
    # allocates 2 psum_mm tiles) - an open group whose pool slot gets
    # reused before it closes would deadlock the PE queue.
    sched = {"active": None}

    def grp_done(g):
        return g["idx"] >= len(g["units"])

    def emit_unit(g):
        g["units"][g["idx"]]()
        g["idx"] += 1
        if grp_done(g):
            if sched["active"] is g:
                sched["active"] = None
        else:
            sched["active"] = g

    def close_active():
        g = sched["active"]
        if g is not None:
            while not grp_done(g):
                emit_unit(g)

    def force_due(i):
        due = [g for g in groups if not grp_done(g) and g["need"] <= i + 1]
        if due:
            close_active()
            for g in due:
                while not grp_done(g):
                    emit_unit(g)

    def pop_one(i):
        g = sched["active"]
        if g is not None:
            emit_unit(g)
            return True
        # acc pairs allocate 2 psum_mm tiles at iter % 16 == LAG + 3 (the
        # delayed sweep-start acc, see acc_q below); an open group must
        # close strictly before that allocation.
        boundary_dist = ((LAG + 3) - (i % 16)) % 16
        if boundary_dist == 0:
            boundary_dist = 16
        for g in groups:
            if grp_done(g) or g["rel"] > i:
                continue
            if len(g["units"]) > boundary_dist:
                continue  # would hold a psum group across a sweep boundary
            emit_unit(g)
            return True
        # hold ~9 Wo groups back for the exp-gated drain + final
        # normalize window at the very end, where the PE otherwise idles
        # (and its clock gate drops to half speed right before the final
        # Wo burst).  Never burst them into the last sweep's steps - that
        # just stalls the exp stream there instead.
        if wo_q and len(wo_q) > 7:
            wo_q.pop(0)()
            return True
        return False

    # Acc emission runs LAG behind the front, except each sweep's first
    # three steps are delayed a further ~3 iters (catching up two accs
    # per iter): the new sweep's acc pair reuses the psum slots of the
    # previous pair, which are only released by its normalize chain
    # (reciprocal -> 2 serial gpsimd broadcasts -> muls, ~4.5us) - with
    # uniform lag that chain stalls the PE at every sweep boundary.
    acc_q = []

    def drain_accs(i, limit=2):
        n = 0
        while acc_q and n < limit:
            ca, pra, sta = acc_q[0]
            # (c0, pr0) has no previous acc pair to wait on: no delay
            if sta == 0 and (ca, pra) != (0, 0) and i < (ca * 2 + pra) * NST + LAG + 3:
                break
            emit_acc(*acc_q.pop(0))
            n += 1

    for i, s in enumerate(steps):
        force_due(i)
        emit_front(*s)
        if i >= LAG:
            acc_q.append(steps[i - LAG])
        drain_accs(i)
        # skip unit pops while catching up on delayed accs (the catch-up
        # iters already carry double acc work)
        if len(acc_q) <= 1:
            pop_one(i)

    close_active()
    for g in groups:
        while not grp_done(g):
            emit_unit(g)
    # the trailing accs are exp-gated (~1.1us apart with the PE mostly
    # idle); interleave half the held-back Wo groups here and leave the
    # rest for the final-chunk normalize window (inside the last
    # emit_acc), keeping the PE busy and its clock gate open throughout
    for j, i in enumerate(range(len(steps), len(steps) + LAG)):
        if wo_q and j < LAG - 2:
            wo_q.pop(0)()
        acc_q.append(steps[i - LAG])
        drain_accs(10**9)
    while wo_q:
        wo_q.pop(0)()


def _build():
    global _BUILT
    if _BUILT is not None:
        return _BUILT
    nc = bacc.Bacc(
        "TRN2",
        target_bir_lowering=False,
        debug=False,
        enable_asserts=False,
        num_devices=8,
    )
    io = {}
    # x tensors are chunk-contiguous: [c][p][k][j] = x^T[k*128+p, c*512+j]
    io["xqT"] = nc.dram_tensor("xqT", [NCH, 128, NK, 512], BF16, kind="ExternalInput").ap()
    io["xkT"] = nc.dram_tensor("xkT", [NCH, 128, NK, 512], BF16, kind="ExternalInput").ap()
    io["xvT"] = nc.dram_tensor("xvT", [NCH, 128, NK, 512], BF16, kind="ExternalInput").ap()
    # weights partition-major: [p][k][j] = W[k*128+p, j]
    io["wq"] = nc.dram_tensor("wq", [128, NK, DQ], BF16, kind="ExternalInput").ap()
    io["wk"] = nc.dram_tensor("wk", [128, NK, DQ], BF16, kind="ExternalInput").ap()
    io["wv"] = nc.dram_tensor("wv", [128, NK, DQ], BF16, kind="ExternalInput").ap()
    io["wo"] = nc.dram_tensor("wo", [128, 2, D], BF16, kind="ExternalInput").ap()
    # biases host-pre-packed: bq/bk as [128, 2] ([p, m] = b[m*128+p]),
    # bv broadcast to [128, DQ]
    io["bq"] = nc.dram_tensor("bq", [128, 2], F32, kind="ExternalInput").ap()
    io["bk"] = nc.dram_tensor("bk", [128, 2], F32, kind="ExternalInput").ap()
    io["bv"] = nc.dram_tensor("bv", [128, DQ], F32, kind="ExternalInput").ap()
    io["out"] = nc.dram_tensor("out", [S, D], BF16, kind="ExternalOutput").ap()
    from contextlib import ExitStack

    with tile.TileContext(nc) as tc, ExitStack() as ctx:
        _emit(ctx, tc, io)
    nc.compile()
    _BUILT = nc
    return nc


def kernel(**inputs):
    global LAST_RESULTS
    bf16 = ml_dtypes.bfloat16
    query = np.asarray(inputs["query"], np.float32).reshape(2, S, D)
    key = np.asarray(inputs["key"], np.float32).reshape(2, S, D)
    value = np.asarray(inputs["value"], np.float32).reshape(2, S, D)
    Wq = np.asarray(inputs["Wq"], np.float32)
    Wk = np.asarray(inputs["Wk"], np.float32)
    Wv = np.asarray(inputs["Wv"], np.float32)
    Wo = np.asarray(inputs["Wo"], np.float32)
    bq = np.asarray(inputs["bq"], np.float32)
    bk = np.asarray(inputs["bk"], np.float32)
    bv = np.asarray(inputs["bv"], np.float32)
    bo = np.asarray(inputs["bo"], np.float32)

    def pack_x(x):
        # [S, D] -> [NCH, 128, NK, 512] with [c,p,k,j] = x.T[k*128+p, c*512+j]
        xt = np.ascontiguousarray(x.T).astype(bf16)
        return np.ascontiguousarray(xt.reshape(NK, 128, NCH, 512).transpose(2, 1, 0, 3))

    def pack_w(w):
        # [D, DQ] -> [128, NK, DQ] with [p,k,j] = w[k*128+p, j]
        return np.ascontiguousarray(w.reshape(NK, 128, DQ).transpose(1, 0, 2).astype(bf16))

    def pack_wo(w):
        # [DQ, D] -> [128, 2, D] with [p,k,j] = w[k*128+p, j]
        return np.ascontiguousarray(w.reshape(2, 128, D).transpose(1, 0, 2).astype(bf16))

    xT = {}
    for b in range(2):
        xT[("q", b)] = pack_x(query[b])
        xT[("k", b)] = pack_x(key[b])
        xT[("v", b)] = pack_x(value[b])

    in_maps = []
    for c in range(8):
        b, g = c // 4, c % 4
        sl = slice(g * DQ, (g + 1) * DQ)
        in_maps.append(
            {
                "xqT": xT[("q", b)],
                "xkT": xT[("k", b)],
                "xvT": xT[("v", b)],
                "wq": pack_w(np.ascontiguousarray(Wq[:, sl])),
                "wk": pack_w(np.ascontiguousarray(Wk[:, sl])),
                "wv": pack_w(np.ascontiguousarray(Wv[:, sl])),
                "wo": pack_wo(np.ascontiguousarray(Wo[sl, :])),
                "bq": np.ascontiguousarray(bq[sl].reshape(2, 128).T),
                "bk": np.ascontiguousarray(bk[sl].reshape(2, 128).T),
                "bv": np.ascontiguousarray(np.broadcast_to(bv[sl][None, :], (128, DQ))),
            }
        )

    nc = _build()
    res = run_bass_kernel_spmd(
        nc, in_maps, core_ids=list(range(8)), trace=TRACE
    )
    LAST_RESULTS = res

    full = np.zeros((2, S, D), np.float32)
    for c in range(8):
        full[c // 4] += res.results[c]["out"].astype(np.float32)
    full += bo[None, None, :]
    return full.reshape(2, S, 1, D)



# revision 83
# speedup vs baseline: 1.0032x; 1.0032x over previous
"""MultiHeadAttention Trainium2 kernel (8 NeuronCores, SPMD).

Sharding: data-parallel over batch (B=2), tensor-parallel over heads
(16 heads -> 4 per core).  Core c handles batch b=c//4, head group
g=c%4 (heads 4g..4g+3).  Wq/Wk/Wv are split column-wise, Wo row-wise;
the per-core Wo partial outputs are summed on the host (replaces the
all-reduce).

Device dataflow per core (bf16 matmuls, f32 PSUM accumulation):
  qT = Wq_g^T x^T   [256, 2048]   (heads on partitions, dh=64 each)
  kT = Wk_g^T x^T   [256, 2048]
  v  = x Wv_g       [2048, 256] stored interleaved with a ones column
                    per head: vaug[st] = [vA|1|vB|1|vC|1|vD|1]
  per (s_q chunk of 512, head pair):
    logitsT[s_k, s_q] = kT^T qT / 8       (two heads packed in PE row
                                           groups, K=64 each)
    p = exp(logitsT)  on ScalarE, scale=1/8 fused, bf16 out
    accT[65, s_q] += vaug_h^T p           (row 64 = softmax denominator)
    outcatT[h] = accT[0:64] * bcast(1/accT[64])   (deferred softmax norm)
  partial = outcatT^T Wo_g  -> DRAM bf16 (summed in f32 on the host)

The kernel is organized as one flat software pipeline: the exp stream
on ScalarE is the pacer (~1.15us per (c,pr,st) step); everything else
(projections, Wo, output DMA) is deadline-scheduled into the PE slack
under it.  Inputs are loaded in 512-column chunks (one 3D DMA
descriptor per chunk) so the first exp fires after ~4.5MB of DMA
instead of the full 14.7MB.
"""

import itertools
import sys

import numpy as np

sys.path.insert(0, "/opt/trn_rl_repo")

import ml_dtypes  # noqa: E402

import concourse.bass as bass  # noqa: E402
import concourse.mybir as mybir  # noqa: E402
import concourse.tile as tile  # noqa: E402
from concourse import bacc  # noqa: E402
from concourse.bass import ts  # noqa: E402
from concourse.bass_utils import run_bass_kernel_spmd  # noqa: E402

S = 2048  # sequence length (S * X)
D = 1024  # model dim
H = 16  # total heads
HL = 4  # heads per core
DH = 64  # head dim
DQ = HL * DH  # per-core projection width = 256
NK = D // 128  # K tiles for projections = 8
NST = S // 128  # s_k tiles = 16
NCH = S // 512  # s_q chunks = 4
LAG = 7  # front-to-accumulate pipeline distance (runahead for stalls)

BF16 = mybir.dt.bfloat16
F32 = mybir.dt.float32

TRACE = False
LAST_RESULTS = None

_BUILT = None


def _emit(ctx, tc, io):
    nc = tc.nc
    xq, xk, xv = io["xqT"], io["xkT"], io["xvT"]
    wq, wk, wv, wo = io["wq"], io["wk"], io["wv"], io["wo"]
    bq, bk, bv = io["bq"], io["bk"], io["bv"]
    out = io["out"]

    consts = ctx.enter_context(tc.tile_pool(name="consts", bufs=1))
    xin = ctx.enter_context(tc.tile_pool(name="xin", bufs=1))
    qk = ctx.enter_context(tc.tile_pool(name="qk", bufs=1))
    ptiles = ctx.enter_context(tc.tile_pool(name="ptiles", bufs=12))
    norm = ctx.enter_context(tc.tile_pool(name="norm", bufs=3))
    osb_pool = ctx.enter_context(tc.tile_pool(name="osb", bufs=4))
    psum_mm = ctx.enter_context(tc.tile_pool(name="psum_mm", bufs=4, space="PSUM"))
    psum_lg = ctx.enter_context(tc.tile_pool(name="psum_lg", bufs=2, space="PSUM"))

    # x and W live as single 3D tiles: [128, k_tile, cols].  One DMA
    # descriptor loads a 512-column chunk of all 8 k-tiles at once (the
    # per-dma_start enqueue cost on the issuing engine is ~650ns, so
    # descriptor count is what paces the input stream).
    wq_all = consts.tile([128, NK, DQ], BF16, tag="wq", name="wq_all")
    wk_all = consts.tile([128, NK, DQ], BF16, tag="wk", name="wk_all")
    wv_all = consts.tile([128, NK, DQ], BF16, tag="wv", name="wv_all")
    wo_all = consts.tile([128, 2, D], BF16, tag="wo", name="wo_all")
    xq_all = xin.tile([128, NK, S], BF16, tag="xq", name="xq_all")
    xk_all = xin.tile([128, NK, S], BF16, tag="xk", name="xk_all")
    xv_all = xin.tile([128, NK, S], BF16, tag="xv", name="xv_all")
    # bq/bk as [128, 2] per-partition scalars (col j = dq 128j..128j+127)
    bq_sb = consts.tile([128, 2], F32, tag="bq", name="bq_sb")
    bk_sb = consts.tile([128, 2], F32, tag="bk", name="bk_sb")
    bv_sb = consts.tile([128, DQ], F32, tag="bv", name="bv_sb")

    # PE warmup: dummy back-to-back matmuls at t=0, while the PE would
    # otherwise sit idle waiting for input DMA.  The PE clock gate (HAM)
    # defaults to 4/8 throttle (1.2 GHz) and only releases after ~3.4us
    # of sustained activity; without this the whole DMA-paced front runs
    # at half clock.
    wu_sb = consts.tile([128, 512], BF16, tag="wu", name="wu_sb")
    nc.vector.memset(wu_sb[:], 1.0)
    wu_ps = psum_lg.tile([128, 512], F32, tag="lg", name="lg")

    for _ in range(48):
        nc.tensor.matmul(wu_ps[:], wu_sb[:, 0:128], wu_sb[:], start=True, stop=True)

    # ---- input DMA.  Host arrays are pre-packed chunk-contiguous
    # ([128 part, k, cols] blocks) so every transfer reads 4-8KB
    # sequential HBM runs per partition.  Three descriptor queues (two
    # HWDGE + gpsimd SWDGE) stream in arrival-priority order: the first
    # (c0, pr0) attention sweep consumes ALL xk chunks plus xq c0/xv c0,
    # so xk is spread across both HWDGE queues ahead of everything else.
    # biases are host-pre-packed to partition-major layouts so these are
    # single-descriptor contiguous transfers (4-byte-run patterns choke
    # the ring for ~7us each)
    nc.sync.dma_start(
        out=bk_sb[:], in_=bass.AP(tensor=bk.tensor, offset=bk.offset, ap=[[2, 128], [1, 2]])
    )
    nc.sync.dma_start(
        out=bq_sb[:], in_=bass.AP(tensor=bq.tensor, offset=bq.offset, ap=[[2, 128], [1, 2]])
    )

    def dma_w(q, dst, src, nkt, cols):
        q.dma_start(
            out=dst[:],
            in_=bass.AP(
                tensor=src.tensor,
                offset=src.offset,
                ap=[[nkt * cols, 128], [cols, nkt], [1, cols]],
            ),
        )

    def dma_chunk(q, dst, src, c):
        q.dma_start(
            out=dst[:, :, ts(c, 512)],
            in_=bass.AP(
                tensor=src.tensor,
                offset=src.offset + c * (128 * NK * 512),
                ap=[[NK * 512, 128], [512, NK], [1, 512]],
            ),
        )

    def dma_half(q, dst, src, c, half):
        q.dma_start(
            out=dst[:, :, c * 512 + half * 256 : c * 512 + half * 256 + 256],
            in_=bass.AP(
                tensor=src.tensor,
                offset=src.offset + c * (128 * NK * 512) + half * 256,
                ap=[[NK * 512, 128], [512, NK], [1, 256]],
            ),
        )

    # Measured DMA behavior: aggregate input bandwidth is hard-capped at
    # ~320-360 GB/s/core; the gpsimd SWDGE stream sustains ~210 GB/s on
    # its own (serial, ucode sprays hw rings) while each HWDGE ring
    # (sync / scalar engines) only gets ~40-50 GB/s under contention.
    # So the arrival-critical chain (xk0, xq0, then the mid-sweep
    # chunks) rides the fast SWDGE stream in consumption order; the
    # 0.5MB weights and latency-tolerant late chunks go on the rings.
    nc.sync.dma_start(
        out=bv_sb[:], in_=bass.AP(tensor=bv.tensor, offset=bv.offset, ap=[[DQ, 128], [1, DQ]])
    )
    dma_chunk(nc.sync, xv_all, xv, 1)
    dma_chunk(nc.sync, xq_all, xq, 1)
    dma_w(nc.scalar, wq_all, wq, NK, DQ)
    dma_chunk(nc.scalar, xk_all, xk, 1)
    dma_chunk(nc.scalar, xv_all, xv, 2)
    dma_chunk(nc.scalar, xq_all, xq, 2)
    dma_w(nc.gpsimd, wk_all, wk, NK, DQ)
    dma_chunk(nc.gpsimd, xk_all, xk, 0)
    dma_chunk(nc.gpsimd, xq_all, xq, 0)
    dma_chunk(nc.gpsimd, xv_all, xv, 0)
    dma_w(nc.gpsimd, wv_all, wv, NK, DQ)
    dma_chunk(nc.gpsimd, xk_all, xk, 2)
    dma_chunk(nc.gpsimd, xk_all, xk, 3)
    dma_chunk(nc.gpsimd, xv_all, xv, 3)
    dma_chunk(nc.gpsimd, xq_all, xq, 3)
    dma_w(nc.gpsimd, wo_all, wo, 2, D)

    # ---- projections: qT, kT = [256, 2048] as 2 tiles of [128, 2048] ----
    qT = [qk.tile([128, S], BF16, tag=f"qT{m}", name=f"qT{m}") for m in range(2)]
    kT = [qk.tile([128, S], BF16, tag=f"kT{m}", name=f"kT{m}") for m in range(2)]

    def qk_quarter(w_all, dst, b_sb, m, c, on_act, state, q):
        """Quarter of a q/k projection PSUM group (k pair 2q, 2q+1).

        Quarters are ~0.43us PE lumps so they interleave between attention
        steps without stalling the exp feed (the front can only run ~2
        steps ahead of the exp stream via the psum_lg double buffer).  The
        PSUM group stays open between quarters; unrelated matmuls to other
        banks interleave freely, but the scheduler bounds how many other
        psum_mm allocations occur while a group is open (pool slot reuse
        of an open group would deadlock the PE queue).
        """
        x_all = xq_all if dst is qT else xk_all
        if q == 0:
            state["ps"] = psum_mm.tile([128, 512], F32, tag="mm", name="mm")
        ps = state["ps"]
        for k in (2 * q, 2 * q + 1):
            nc.tensor.matmul(
                ps[:],
                w_all[:, k, ts(m, 128)],
                x_all[:, k, ts(c, 512)],
                start=(k == 0),
                stop=(k == NK - 1),
            )
        if q == 3:
            if on_act:  # prefix phase: ScalarE is idle there
                nc.scalar.add(dst[m][:, ts(c, 512)], ps[:], b_sb[:, m : m + 1])
            else:  # injected into attention: keep ScalarE free for exp
                nc.vector.tensor_scalar_add(dst[m][:, ts(c, 512)], ps[:], b_sb[:, m : m + 1])

    # vaug[st] = [vA|1|vB|1|vC|1|vD|1]  [128, 260]
    vaug = [qk.tile([128, HL * (DH + 1)], BF16, tag=f"vaug{st}", name=f"vaug{st}") for st in range(NST)]

    # the ones-columns of every vaug tile are set up front, during the
    # DMA-wait window where the DVE is idle (the per-head adds below
    # only write the value columns)
    for st in range(NST):
        nc.vector.memset(vaug[st][:], 1.0)

    def v_half(st, state, h):
        """Half of the v projection for one st tile (k 0-3 or 4-7)."""
        if h == 0:
            state["ps"] = psum_mm.tile([128, DQ], F32, tag="mm", name="mm")
        ps = state["ps"]
        for k in range(4 * h, 4 * h + 4):
            nc.tensor.matmul(
                ps[:],
                xv_all[:, k, ts(st, 128)],
                wv_all[:, k, :],
                start=(k == 0),
                stop=(k == NK - 1),
            )
        if h == 1:
            for hd in range(HL):
                nc.vector.tensor_add(
                    vaug[st][:, hd * 65 : hd * 65 + 64],
                    ps[:, ts(hd, DH)],
                    bv_sb[:, ts(hd, DH)],
                )

    octT = [qk.tile([128, S], BF16, tag=f"octT{m}", name=f"octT{m}") for m in range(2)]
    store_q = itertools.cycle([nc.sync, nc.gpsimd])
    tail_store_q = itertools.cycle([nc.sync, nc.scalar])
    tail_cp = itertools.cycle([True, False])  # alternate scalar/vector copies

    tail_mode = {"on": False}  # set once the exp stream is fully drained
    wo_psum = itertools.cycle([0, 1, 0])  # tail: spread over mm + lg banks

    def wo_group(c, smt, ncho, on_act=False):
        row = c * 512 + smt * 128
        if tail_mode["on"] and next(wo_psum):
            # the exp stream is done, so the logits banks are free: using
            # them decouples the final Wo matmuls from the copy drain
            ps = psum_lg.tile([128, 512], F32, tag="lg", name="lg")
        else:
            ps = psum_mm.tile([128, 512], F32, tag="mm", name="mm")
        for k in range(2):
            nc.tensor.matmul(
                ps[:],
                octT[k][:, row : row + 128],
                wo_all[:, k, ts(ncho, 512)],
                start=(k == 0),
                stop=(k == 1),
            )
        osb = osb_pool.tile([128, 512], BF16, tag="osb", name="osb")
        if (on_act or tail_mode["on"]) and next(tail_cp):
            # tail drain: split copies between the idle ScalarE and DVE
            nc.scalar.copy(osb[:], ps[:])
        else:
            nc.vector.tensor_copy(osb[:], ps[:])
        # tail stores avoid the SWDGE queue: its end-of-kernel drain
        # (~3us) starts only after its last descriptor completes.  They
        # alternate across both HWDGE rings so the final burst drains in
        # parallel.
        q = next(tail_store_q) if tail_mode["on"] else next(store_q)
        q.dma_start(out[row : row + 128, ts(ncho, 512)], osb[:])

    # ---- prefix: just enough for the first exp: kT/qT chunk 0 of the
    # pr0 pair.  Everything else is scheduled into the attention
    # pipeline below, one ~0.43us quarter at a time, ordered by DMA
    # arrival estimate and forced by consumption deadline.
    st0 = {}
    for q in range(4):
        qk_quarter(wk_all, kT, bk_sb, 0, 0, True, st0, q)
    # keep the PE clock gate open while the qT prefix waits for xq0/wq
    for _ in range(18):
        nc.tensor.matmul(wu_ps[:, 0:256], wu_sb[:, 0:128], wu_sb[:, 0:256], start=True, stop=True)
    st1 = {}
    for q in range(4):
        qk_quarter(wq_all, qT, bq_sb, 0, 0, True, st1, q)

    steps = [(c, pr, st) for c in range(NCH) for pr in range(2) for st in range(NST)]

    # Arrival estimates (us from kernel start) for each input buffer,
    # from the queue FIFOs above: SWDGE ~210 GB/s serial starting ~7.5us,
    # HWDGE rings ~70 GB/s each.  The step->time map is piecewise: the
    # first two sweeps are dense with forced projection units (~1.7
    # us/step), after which the pipeline cruises at ~1.15 us/step.
    T0_US = 23.0
    ARR_XK = [19.0, 26.0, 38.0, 42.0]
    ARR_XQ = [24.0, 47.0, 56.0, 52.0]
    ARR_XV = [29.5, 33.0, 41.0, 47.0]

    def astep(a_us):
        t = a_us - T0_US
        if t <= 0:
            return 0
        if t <= 32 * 1.75:
            return int(t / 1.75) + 1
        return 32 + int((t - 32 * 1.75) / 1.15) + 1

    # Work groups: each is a sequence of ~0.43us PE units sharing one
    # open psum_mm accumulation group.  rel = earliest emission step
    # (DMA arrival), need = step whose front/acc consumes the output.
    groups = []

    def add_group(rel_us, need, units):
        groups.append({"rel": astep(rel_us), "need": need, "units": units, "idx": 0})

    def qk_units(w_all, dst, b_sb, m, c):
        state = {}
        return [
            (lambda w=w_all, d=dst, b=b_sb, mm=m, cc=c, s=state, q=q: qk_quarter(w, d, b, mm, cc, False, s, q))
            for q in range(4)
        ]

    def v_units(st):
        state = {}
        return [(lambda s=st, ss=state, h=h: v_half(s, ss, h)) for h in range(2)]

    for st in range(NST):  # vaug[st] consumed at acc step st (= iter st+LAG)
        add_group(ARR_XV[st // 4], st + LAG, v_units(st))
    for c in range(1, NCH):  # kT m0 chunk c consumed at front step 4c
        add_group(ARR_XK[c], 4 * c, qk_units(wk_all, kT, bk_sb, 0, c))
    add_group(ARR_XQ[0], 16, qk_units(wq_all, qT, bq_sb, 1, 0))
    for c in range(NCH):  # kT m1 chunk c consumed at front step 16+4c
        add_group(ARR_XK[c], 16 + 4 * c, qk_units(wk_all, kT, bk_sb, 1, c))
    for c in range(1, NCH):  # qT m chunk c consumed from front step 32c+16m
        add_group(ARR_XQ[c], 32 * c, qk_units(wq_all, qT, bq_sb, 0, c))
        add_group(ARR_XQ[c], 32 * c + 16, qk_units(wq_all, qT, bq_sb, 1, c))
    groups.sort(key=lambda g: g["need"])

    wo_q = []
    acc_map = {}
    p_map = {}

    def emit_front(c, pr, st):
        lg = psum_lg.tile([128, 1024], F32, tag="lg", name="lg")
        for hh in range(2):
            nc.tensor.matmul(
                lg[:, ts(hh, 512)],
                kT[pr][ts(hh, 64), ts(st, 128)],
                qT[pr][ts(hh, 64), ts(c, 512)],
                start=True,
                stop=True,
            )
        p = ptiles.tile([128, 1024], BF16, tag="p", name="p")
        nc.scalar.activation(p[:], lg[:], mybir.ActivationFunctionType.Exp, scale=0.125)
        p_map[(c, pr, st)] = p

    def emit_acc(c, pr, st):
        if st == 0:
            acc_map[(c, pr)] = [
                psum_mm.tile([65, 512], F32, tag="mm", name="mm") for _ in range(2)
            ]
        acc = acc_map[(c, pr)]
        pp = p_map.pop((c, pr, st))
        for hh in range(2):
            h = 2 * pr + hh
            nc.tensor.matmul(
                acc[hh][:],
                vaug[st][:, h * 65 : h * 65 + 65],
                pp[:, ts(hh, 512)],
                start=(st == 0),
                stop=(st == NST - 1),
            )
        if st == NST - 1:
            # normalize: octT[pr][64*hh, chunk c] = acc[0:64] / acc[64].
            # Reciprocal on the [1,512] denominator row, gpsimd broadcast
            # to 64 partitions, DVE muls straight out of acc PSUM.
            final = c == NCH - 1 and pr == 1
            bcs = []
            for hh in range(2):
                den = norm.tile([1, 512], F32, tag="den", name="den")
                if final and hh == 1:
                    # the exp stream is complete by now; ScalarE can take
                    # one den copy so the two chains start in parallel
                    nc.scalar.copy(den[:], acc[hh][64:65, :])
                else:
                    nc.vector.tensor_copy(den[:], acc[hh][64:65, :])
                rden = norm.tile([1, 512], F32, tag="rden", name="rden")
                nc.vector.reciprocal_approx_fast(rden[:], den[:])
                bc = norm.tile([64, 512], F32, tag="bcs", name="bcs")
                nc.gpsimd.partition_broadcast(bc[:], rden[:])
                bcs.append(bc)
            if final:
                # PE fill during the reciprocal/broadcast chain: drain the
                # held-back Wo backlog (also keeps the clock gate open for
                # the final Wo burst below).  The exp stream is complete
                # here: ScalarE takes copies and the lg banks join the
                # Wo psum rotation.
                tail_mode["on"] = True
                while wo_q:
                    wo_q.pop(0)()
                for smt in range(4):
                    for hh in range(2):
                        nc.vector.tensor_mul(
                            octT[pr][ts(hh, 64), c * 512 + smt * 128 : c * 512 + smt * 128 + 128],
                            acc[hh][0:64, ts(smt, 128)],
                            bcs[hh][:, ts(smt, 128)],
                        )
                    wo_group(c, smt, 0, on_act=True)
                    wo_group(c, smt, 1, on_act=True)
            else:
                for hh in range(2):
                    nc.vector.tensor_mul(
                        octT[pr][ts(hh, 64), ts(c, 512)], acc[hh][0:64, :], bcs[hh][:]
                    )
            del acc_map[(c, pr)]
            if pr == 1 and not final:
                wo_q.extend(
                    (lambda cc=c, smt=smt, ncho=ncho: wo_group(cc, smt, ncho))
                    for smt in range(4)
                    for ncho in range(2)
                )

    # Scheduler state: `active` is the group currently being dribbled
    # out one unit per step.  While a psum group is open we emit ONLY
    # its units (no other psum_mm allocations), and we never open a
    # 4-unit group within 3 steps of a sweep boundary (emit_acc st==0
    # allocates 2 psum_mm tiles) - an open group whose pool slot gets
    # reused before it closes would deadlock the PE queue.
    sched = {"active": None}

    def grp_done(g):
        return g["idx"] >= len(g["units"])

    def emit_unit(g):
        g["units"][g["idx"]]()
        g["idx"] += 1
        if grp_done(g):
            if sched["active"] is g:
                sched["active"] = None
        else:
            sched["active"] = g

    def close_active():
        g = sched["active"]
        if g is not None:
            while not grp_done(g):
                emit_unit(g)

    def force_due(i):
        due = [g for g in groups if not grp_done(g) and g["need"] <= i + 1]
        if due:
            close_active()
            for g in due:
                while not grp_done(g):
                    emit_unit(g)

    def pop_one(i):
        g = sched["active"]
        if g is not None:
            emit_unit(g)
            return True
        # acc pairs allocate 2 psum_mm tiles at iter % 16 == LAG + 3 (the
        # delayed sweep-start acc, see acc_q below); an open group must
        # close strictly before that allocation.
        boundary_dist = ((LAG + 3) - (i % 16)) % 16
        if boundary_dist == 0:
            boundary_dist = 16
        for g in groups:
            if grp_done(g) or g["rel"] > i:
                continue
            if len(g["units"]) > boundary_dist:
                continue  # would hold a psum group across a sweep boundary
            emit_unit(g)
            return True
        # hold ~9 Wo groups back for the exp-gated drain + final
        # normalize window at the very end, where the PE otherwise idles
        # (and its clock gate drops to half speed right before the final
        # Wo burst).  Never burst them into the last sweep's steps - that
        # just stalls the exp stream there instead.
        if wo_q and len(wo_q) > 9:
            wo_q.pop(0)()
            return True
        return False

    # Acc emission runs LAG behind the front, except each sweep's first
    # three steps are delayed a further ~3 iters (catching up two accs
    # per iter): the new sweep's acc pair reuses the psum slots of the
    # previous pair, which are only released by its normalize chain
    # (reciprocal -> 2 serial gpsimd broadcasts -> muls, ~4.5us) - with
    # uniform lag that chain stalls the PE at every sweep boundary.
    acc_q = []

    def drain_accs(i, limit=2):
        n = 0
        while acc_q and n < limit:
            ca, pra, sta = acc_q[0]
            # (c0, pr0) has no previous acc pair to wait on: no delay
            if sta == 0 and (ca, pra) != (0, 0) and i < (ca * 2 + pra) * NST + LAG + 3:
                break
            emit_acc(*acc_q.pop(0))
            n += 1

    for i, s in enumerate(steps):
        force_due(i)
        emit_front(*s)
        if i >= LAG:
            acc_q.append(steps[i - LAG])
        drain_accs(i)
        # skip unit pops while catching up on delayed accs (the catch-up
        # iters already carry double acc work)
        if len(acc_q) <= 1:
            pop_one(i)

    close_active()
    for g in groups:
        while not grp_done(g):
            emit_unit(g)
    # the trailing accs are exp-gated (~1.1us apart with the PE mostly
    # idle); interleave half the held-back Wo groups here and leave the
    # rest for the final-chunk normalize window (inside the last
    # emit_acc), keeping the PE busy and its clock gate open throughout
    for j, i in enumerate(range(len(steps), len(steps) + LAG)):
        if wo_q and j < LAG - 2:
            wo_q.pop(0)()
        acc_q.append(steps[i - LAG])
        drain_accs(10**9)
    while wo_q:
        wo_q.pop(0)()


def _build():
    global _BUILT
    if _BUILT is not None:
        return _BUILT
    nc = bacc.Bacc(
        "TRN2",
        target_bir_lowering=False,
        debug=False,
        enable_asserts=False,
        num_devices=8,
    )
    io = {}
    # x tensors are chunk-contiguous: [c][p][k][j] = x^T[k*128+p, c*512+j]
    io["xqT"] = nc.dram_tensor("xqT", [NCH, 128, NK, 512], BF16, kind="ExternalInput").ap()
    io["xkT"] = nc.dram_tensor("xkT", [NCH, 128, NK, 512], BF16, kind="ExternalInput").ap()
    io["xvT"] = nc.dram_tensor("xvT", [NCH, 128, NK, 512], BF16, kind="ExternalInput").ap()
    # weights partition-major: [p][k][j] = W[k*128+p, j]
    io["wq"] = nc.dram_tensor("wq", [128, NK, DQ], BF16, kind="ExternalInput").ap()
    io["wk"] = nc.dram_tensor("wk", [128, NK, DQ], BF16, kind="ExternalInput").ap()
    io["wv"] = nc.dram_tensor("wv", [128, NK, DQ], BF16, kind="ExternalInput").ap()
    io["wo"] = nc.dram_tensor("wo", [128, 2, D], BF16, kind="ExternalInput").ap()
    # biases host-pre-packed: bq/bk as [128, 2] ([p, m] = b[m*128+p]),
    # bv broadcast to [128, DQ]
    io["bq"] = nc.dram_tensor("bq", [128, 2], F32, kind="ExternalInput").ap()
    io["bk"] = nc.dram_tensor("bk", [128, 2], F32, kind="ExternalInput").ap()
    io["bv"] = nc.dram_tensor("bv", [128, DQ], F32, kind="ExternalInput").ap()
    io["out"] = nc.dram_tensor("out", [S, D], BF16, kind="ExternalOutput").ap()
    from contextlib import ExitStack

    with tile.TileContext(nc) as tc, ExitStack() as ctx:
        _emit(ctx, tc, io)
    nc.compile()
    _BUILT = nc
    return nc


def kernel(**inputs):
    global LAST_RESULTS
    bf16 = ml_dtypes.bfloat16
    query = np.asarray(inputs["query"], np.float32).reshape(2, S, D)
    key = np.asarray(inputs["key"], np.float32).reshape(2, S, D)
    value = np.asarray(inputs["value"], np.float32).reshape(2, S, D)
    Wq = np.asarray(inputs["Wq"], np.float32)
    Wk = np.asarray(inputs["Wk"], np.float32)
    Wv = np.asarray(inputs["Wv"], np.float32)
    Wo = np.asarray(inputs["Wo"], np.float32)
    bq = np.asarray(inputs["bq"], np.float32)
    bk = np.asarray(inputs["bk"], np.float32)
    bv = np.asarray(inputs["bv"], np.float32)
    bo = np.asarray(inputs["bo"], np.float32)

    def pack_x(x):
        # [S, D] -> [NCH, 128, NK, 512] with [c,p,k,j] = x.T[k*128+p, c*512+j]
        xt = np.ascontiguousarray(x.T).astype(bf16)
        return np.ascontiguousarray(xt.reshape(NK, 128, NCH, 512).transpose(2, 1, 0, 3))

    def pack_w(w):
        # [D, DQ] -> [128, NK, DQ] with [p,k,j] = w[k*128+p, j]
        return np.ascontiguousarray(w.reshape(NK, 128, DQ).transpose(1, 0, 2).astype(bf16))

    def pack_wo(w):
        # [DQ, D] -> [128, 2, D] with [p,k,j] = w[k*128+p, j]
        return np.ascontiguousarray(w.reshape(2, 128, D).transpose(1, 0, 2).astype(bf16))

    xT = {}
    for b in range(2):
        xT[("q", b)] = pack_x(query[b])
        xT[("k", b)] = pack_x(key[b])
        xT[("v", b)] = pack_x(value[b])

    in_maps = []
    for c in range(8):
        b, g = c // 4, c % 4
        sl = slice(g * DQ, (g + 1) * DQ)
        in_maps.append(
            {
                "xqT": xT[("q", b)],
                "xkT": xT[("k", b)],
                "xvT": xT[("v", b)],
                "wq": pack_w(np.ascontiguousarray(Wq[:, sl])),
                "wk": pack_w(np.ascontiguousarray(Wk[:, sl])),
                "wv": pack_w(np.ascontiguousarray(Wv[:, sl])),
                "wo": pack_wo(np.ascontiguousarray(Wo[sl, :])),
                "bq": np.ascontiguousarray(bq[sl].reshape(2, 128).T),
                "bk": np.ascontiguousarray(bk[sl].reshape(2, 128).T),
                "bv": np.ascontiguousarray(np.broadcast_to(bv[sl][None, :], (128, DQ))),
            }
        )

    nc = _build()
    res = run_bass_kernel_spmd(
        nc, in_maps, core_ids=list(range(8)), trace=TRACE
    )
    LAST_RESULTS = res

    full = np.zeros((2, S, D), np.float32)
    for c in range(8):
        full[c // 4] += res.results[c]["out"].astype(np.float32)
    full += bo[None, None, :]
    return full.reshape(2, S, 1, D)

